# revision 19
# baseline (speedup 1.0000x reference)
"""Dense graph-attention layer (GAT) on 8 Trainium2 NeuronCores.

Sharding: data-parallel over batch B=8 -> one batch element per core.
Adjacency mask and per-head params are replicated.

Per-core math (b fixed), derived from the reference:
  proj_h   = x_b @ W_h + bias_h                      [N, O]
  src_h[j] = proj_h[j] . attn_src_h  = x'_b[j] . (W'_h @ attn_src_h)
  dst_h[i] = proj_h[i] . attn_dst_h
  logit[i,j] = leaky_relu(dst_i + src_j, 0.2) + softplus(beta_h)*prior_b[j]
  att = masked softmax over j;  out_i = sum_j att[i,j] proj_h[j]

Key identities used on device:
  - exp(leaky(t) + p) = max(exp(t + p), exp(0.2 t + p))  (exp monotonic)
  - softmax shift-invariance: no row-max subtraction needed (logits <= ~40,
    exp never overflows fp32); masked entries get t-=1e30 so exp -> 0.
  - denominator = extra all-ones column appended to proj (matmul computes
    both the weighted sum and the normalizer in one accumulation).

Device layout per core ("T" = transposed, j on partitions):
  Et tile [128 j, 1024 i] = exp-scores; aggregation matmul contracts j on
  partitions: out[128 i, 33] += Et[:, i-tile].T @ proj''[j-tile, 33].
"""

import numpy as np

import concourse.bass as bass
import concourse.tile as tile
from concourse import bacc, mybir
from concourse.bass_utils import run_bass_kernel_spmd


def _install_ntff_shim():
    """Provide antenv.axon_hooks if the image lacks it, wiring the NTFF
    profile hook to libaxon_pjrt.so so trace=True runs can report HW time."""
    try:
        import antenv.axon_hooks  # noqa: F401

        return
    except ImportError:
        pass
    try:
        import sys
        import types

        import antenv

        mod = types.ModuleType("antenv.axon_hooks")
        state = {"hook": None}
        mod.set_axon_ntff_profile_hook = lambda h: state.__setitem__("hook", h)
        mod.get_axon_ntff_profile_hook = lambda: state["hook"]
        sys.modules["antenv.axon_hooks"] = mod
        antenv.axon_hooks = mod
        try:
            from trn_agent_boot.trn_boot import _ntff_profile_via_ctypes

            hook = _ntff_profile_via_ctypes("/opt/axon/libaxon_pjrt.so")
            if hook is not None:
                mod.set_axon_ntff_profile_hook(hook)
        except Exception:
            pass
    except Exception:
        pass


_install_ntff_shim()

B, N, IDIM, O, H = 8, 1024, 64, 32, 4
NT = N // 128  # 8 partition tiles
OC = O + 1  # proj columns + ones column (denominator)
F32 = mybir.dt.float32
NEG_BIG = -1.0e30

_NC_CACHE = {}


def _build_nc():
    # Bacc: its finalize() runs move_matmul_waits_to_ldweights +
    # generate_event_semaphores, which legalize multi-wait instructions
    # (PE Matmult has a single hardware wait slot).
    nc = bacc.Bacc()
    WC = H * OC  # 132
    # cst = [xT | wcat | adstb] merged so one DMA (one queue semaphore)
    # covers every matmul input -- PE LDWEIGHTS has a single wait slot.
    CW = N + (WC + H) + H * 128
    cst = nc.declare_dram_parameter("cst", [IDIM + 1, CW], F32, isOutput=False)
    adjm = nc.declare_dram_parameter("adjm", [128, NT * N], F32, isOutput=False)
    pcol = nc.declare_dram_parameter("pcol", [128, NT * H], F32, isOutput=False)
    out = nc.declare_dram_parameter("out", [N, H * O], F32, isOutput=True)

    Add = mybir.AluOpType.add
    Exp = mybir.ActivationFunctionType.Exp

    with tile.TileContext(nc) as tc:
        with (
            tc.tile_pool(name="consts", bufs=1) as consts,
            tc.tile_pool(name="tmp", bufs=3) as tmp,
            tc.tile_pool(name="epool", bufs=3) as epool,
            tc.tile_pool(name="attp", bufs=10) as attp,
            tc.tile_pool(name="prep", bufs=3, space="PSUM") as prep,
            tc.tile_pool(name="accp", bufs=1, space="PSUM") as accp,
        ):
            sb_cst = consts.tile([IDIM + 1, CW], F32)
            nc.sync.dma_start(out=sb_cst, in_=cst[:, :])
            sb_xT = sb_cst[:, 0:N]
            sb_wcat = sb_cst[:, N : N + WC + H]
            sb_adstb = sb_cst[:, N + WC + H : CW]
            sb_pcol = consts.tile([128, NT * H], F32)
            nc.sync.dma_start(out=sb_pcol, in_=pcol[:, :])
            sb_adjm = consts.tile([128, NT * N], F32)
            for jt in range(NT):
                nc.sync.dma_start(
                    out=sb_adjm[:, jt * N : (jt + 1) * N],
                    in_=adjm[:, jt * N : (jt + 1) * N],
                )
            sb_proj = consts.tile([128, NT * WC], F32)
            sb_src = consts.tile([128, NT * H], F32)
            sb_dstB = consts.tile([128, H * N], F32)
            out_sb = consts.tile([128, N], F32)

            # --- precompute: proj'' (with bias + ones col) and src scores ---
            for jt in range(NT):
                pp = prep.tile([128, 512], F32, tag="pre")
                nc.tensor.matmul(
                    pp[:, : WC + H],
                    lhsT=sb_xT[:, jt * 128 : (jt + 1) * 128],
                    rhs=sb_wcat[:, :],
                    start=True,
                    stop=True,
                )
                nc.vector.tensor_copy(
                    out=sb_proj[:, jt * WC : (jt + 1) * WC], in_=pp[:, :WC]
                )
                nc.vector.tensor_copy(
                    out=sb_src[:, jt * H : (jt + 1) * H], in_=pp[:, WC : WC + H]
                )

            # --- dst scores broadcast across partitions: lhsT columns are the
            # same a_dst vector repeated 128x, so out[m, n] = dst[n] for all m.
            for h in range(H):
                for half in range(2):
                    pb = prep.tile([128, 512], F32, tag="pre")
                    nc.tensor.matmul(
                        pb[:, :],
                        lhsT=sb_adstb[:, h * 128 : (h + 1) * 128],
                        rhs=sb_xT[:, half * 512 : (half + 1) * 512],
                        start=True,
                        stop=True,
                    )
                    nc.vector.tensor_copy(
                        out=sb_dstB[:, h * N + half * 512 : h * N + (half + 1) * 512],
                        in_=pb[:, :],
                    )

            # --- main loop: per (head, j-tile) build Et [128 j, 1024 i] ---
            accs = [
                accp.tile([128, 2 * WC], F32, tag=f"acc{i}", name=f"acc{i}")
                for i in range(4)
            ]
            for h in range(H):
                atts = []
                for jt in range(NT):
                    sc = sb_src[:, jt * H + h : jt * H + h + 1]
                    pc = sb_pcol[:, jt * H + h : jt * H + h + 1]
                    tm = tmp.tile([128, N], F32, tag="tm")
                    # tm = dst_i (bcast rows) + src_j (per-partition) + mask
                    nc.vector.scalar_tensor_tensor(
                        out=tm,
                        in0=sb_dstB[:, h * N : (h + 1) * N],
                        scalar=sc,
                        in1=sb_adjm[:, jt * N : (jt + 1) * N],
                        op0=Add,
                        op1=Add,
                    )
                    e1 = epool.tile([128, N], F32, tag="e1")
                    nc.scalar.activation(out=e1, in_=tm, func=Exp, bias=pc, scale=1.0)
                    e2 = epool.tile([128, N], F32, tag="e2")
                    nc.scalar.activation(out=e2, in_=tm, func=Exp, bias=pc, scale=0.2)
                    att = attp.tile([128, N], F32, tag="att", name=f"att_h{h}_j{jt}")
                    nc.vector.tensor_max(out=att, in0=e1, in1=e2)
                    atts.append(att)
                # it-major, jt reversed: the group leader reads the
                # latest-finishing att tile, so its single DVE wait covers
                # every att and the remaining matmuls carry no sync waits
                # (walrus LW structs have very few wait slots).
                for it in range(NT):
                    acc = accs[it // 2]
                    cb = (it % 2) * WC + h * OC
                    for jt in reversed(range(NT)):
                        nc.tensor.matmul(
                            acc[:, cb : cb + OC],
                            lhsT=atts[jt][:, it * 128 : (it + 1) * 128],
                            rhs=sb_proj[:, jt * WC + h * OC : jt * WC + (h + 1) * OC],
                            start=(jt == NT - 1),
                            stop=(jt == 0),
                        )

            # --- finalize: divide by the ones-column sum, store ---
            for it in range(NT):
                for h in range(H):
                    acc = accs[it // 2]
                    cb = (it % 2) * WC + h * OC
                    d = tmp.tile([128, 1], F32, tag="d")
                    nc.vector.reciprocal(out=d, in_=acc[:, cb + O : cb + O + 1])
                    nc.vector.tensor_scalar_mul(
                        out=out_sb[:, it * 128 + h * O : it * 128 + (h + 1) * O],
                        in0=acc[:, cb : cb + O],
                        scalar1=d,
                    )
            for it in range(NT):
                nc.sync.dma_start(
                    out=out[it * 128 : (it + 1) * 128, :],
                    in_=out_sb[:, it * 128 : (it + 1) * 128],
                )
    nc.finalize()
    return nc


def _get_nc():
    if "nc" not in _NC_CACHE:
        _NC_CACHE["nc"] = _build_nc()
    return _NC_CACHE["nc"]


def _prep_inputs(x, adj, source_prior, beta, weight, attn_src, attn_dst, bias):
    x = np.asarray(x, np.float32)
    adj = np.asarray(adj)
    source_prior = np.asarray(source_prior, np.float32)
    beta = np.asarray(beta, np.float32)
    weight = np.asarray(weight, np.float32)
    attn_src = np.asarray(attn_src, np.float32)
    attn_dst = np.asarray(attn_dst, np.float32)
    bias = np.asarray(bias, np.float32)

    # additive mask, transposed (source j on rows), tiled to [128, NT*N]
    madd = np.where(adj.T != 0, np.float32(0.0), np.float32(NEG_BIG))
    adjm = np.ascontiguousarray(
        madd.reshape(NT, 128, N).transpose(1, 0, 2).reshape(128, NT * N)
    )

    WC = H * OC
    wcat = np.zeros((IDIM + 1, WC + H), np.float32)
    adstb = np.zeros((IDIM + 1, H * 128), np.float32)
    for h in range(H):
        wcat[:IDIM, h * OC : h * OC + O] = weight[h]
        wcat[IDIM, h * OC : h * OC + O] = bias[h]
        wcat[IDIM, h * OC + O] = 1.0  # ones column -> softmax denominator
        wcat[:IDIM, WC + h] = weight[h] @ attn_src[h]
        wcat[IDIM, WC + h] = bias[h] @ attn_src[h]
        a_dst = np.concatenate([weight[h] @ attn_dst[h], bias[h] @ attn_dst[h][:, None]])
        adstb[:, h * 128 : (h + 1) * 128] = a_dst[:, None]

    gain = np.logaddexp(0.0, beta).astype(np.float32)  # softplus

    in_maps = []
    for b in range(B):
        xT = np.ones((IDIM + 1, N), np.float32)
        xT[:IDIM] = x[b].T
        cst = np.ascontiguousarray(np.concatenate([xT, wcat, adstb], axis=1))
        p = gain[None, :] * source_prior[b][:, None]  # [N, H]
        pcol = np.ascontiguousarray(
            p.reshape(NT, 128, H).transpose(1, 0, 2).reshape(128, NT * H)
        )
        in_maps.append({"cst": cst, "adjm": adjm, "pcol": pcol})
    return in_maps


def _run(inputs, trace=False):
    in_maps = _prep_inputs(**inputs)
    nc = _get_nc()
    res = run_bass_kernel_spmd(nc, in_maps, list(range(B)), trace=trace)
    out = np.stack([res.results[b]["out"] for b in range(B)]).astype(np.float32)
    return out, res


def kernel(**inputs):
    out, _ = _run(inputs, trace=False)
    return out


# revision 22
# speedup vs baseline: 1.6969x; 1.6969x over previous
"""Dense graph-attention layer (GAT) on 8 Trainium2 NeuronCores.

Sharding: data-parallel over batch B=8 -> one batch element per core.
Adjacency mask and per-head params are replicated.

Per-core math (b fixed), derived from the reference:
  proj_h   = x_b @ W_h + bias_h                      [N, O]
  src_h[j] = proj_h[j] . attn_src_h  = x'_b[j] . (W'_h @ attn_src_h)
  dst_h[i] = proj_h[i] . attn_dst_h
  logit[i,j] = leaky_relu(dst_i + src_j, 0.2) + softplus(beta_h)*prior_b[j]
  att = masked softmax over j;  out_i = sum_j att[i,j] proj_h[j]

Key identities used on device:
  - exp(leaky(t) + p) = max(exp(t + p), exp(0.2 t + p))  (exp monotonic)
  - softmax shift-invariance: no row-max subtraction needed (logits <= ~40,
    exp never overflows fp32); masked entries get t-=1e30 so exp -> 0.
  - denominator = extra all-ones column appended to proj (matmul computes
    both the weighted sum and the normalizer in one accumulation).

Device layout per core ("T" = transposed, j on partitions):
  Et tile [128 j, 1024 i] = exp-scores; aggregation matmul contracts j on
  partitions: out[128 i, 33] += Et[:, i-tile].T @ proj''[j-tile, 33].
"""

import numpy as np

import concourse.bass as bass
import concourse.tile as tile
from concourse import bacc, mybir
from concourse.bass_utils import run_bass_kernel_spmd


def _install_ntff_shim():
    """Provide antenv.axon_hooks if the image lacks it, wiring the NTFF
    profile hook to libaxon_pjrt.so so trace=True runs can report HW time."""
    try:
        import antenv.axon_hooks  # noqa: F401

        return
    except ImportError:
        pass
    try:
        import sys
        import types

        import antenv

        mod = types.ModuleType("antenv.axon_hooks")
        state = {"hook": None}
        mod.set_axon_ntff_profile_hook = lambda h: state.__setitem__("hook", h)
        mod.get_axon_ntff_profile_hook = lambda: state["hook"]
        sys.modules["antenv.axon_hooks"] = mod
        antenv.axon_hooks = mod
        try:
            from trn_agent_boot.trn_boot import _ntff_profile_via_ctypes

            hook = _ntff_profile_via_ctypes("/opt/axon/libaxon_pjrt.so")
            if hook is not None:
                mod.set_axon_ntff_profile_hook(hook)
        except Exception:
            pass
    except Exception:
        pass


_install_ntff_shim()

B, N, IDIM, O, H = 8, 1024, 64, 32, 4
NT = N // 128  # 8 partition tiles
OC = O + 1  # proj columns + ones column (denominator)
F32 = mybir.dt.float32
BF16 = mybir.dt.bfloat16
NEG_BIG = -1.0e30

_NC_CACHE = {}


def _build_nc():
    # Bacc: its finalize() runs move_matmul_waits_to_ldweights +
    # generate_event_semaphores, which legalize multi-wait instructions
    # (PE Matmult has a single hardware wait slot).
    nc = bacc.Bacc()
    WC = H * OC  # 132
    # cst = [xT | wcat | adstb] merged so one DMA (one queue semaphore)
    # covers every matmul input -- PE LDWEIGHTS has a single wait slot.
    CW = N + (WC + H) + H * 128
    cst = nc.declare_dram_parameter("cst", [IDIM + 1, CW], F32, isOutput=False)
    adjm = nc.declare_dram_parameter("adjm", [128, NT * N], F32, isOutput=False)
    pcol = nc.declare_dram_parameter("pcol", [128, NT * H], F32, isOutput=False)
    out = nc.declare_dram_parameter("out", [N, H * O], F32, isOutput=True)

    Add = mybir.AluOpType.add
    Exp = mybir.ActivationFunctionType.Exp

    with tile.TileContext(nc) as tc:
        with (
            tc.tile_pool(name="consts", bufs=1) as consts,
            tc.tile_pool(name="tmp", bufs=3) as tmp,
            tc.tile_pool(name="epool", bufs=3) as epool,
            tc.tile_pool(name="attp", bufs=10) as attp,
            tc.tile_pool(name="prep", bufs=3, space="PSUM") as prep,
            tc.tile_pool(name="accp", bufs=1, space="PSUM") as accp,
        ):
            sb_cst = consts.tile([IDIM + 1, CW], F32)
            nc.sync.dma_start(out=sb_cst, in_=cst[:, :])
            sb_xT = sb_cst[:, 0:N]
            sb_wcat = sb_cst[:, N : N + WC + H]
            sb_adstb = sb_cst[:, N + WC + H : CW]
            sb_pcol = consts.tile([128, NT * H], F32)
            nc.sync.dma_start(out=sb_pcol, in_=pcol[:, :])
            sb_adjm = consts.tile([128, NT * N], F32)
            for jt in range(NT):
                nc.sync.dma_start(
                    out=sb_adjm[:, jt * N : (jt + 1) * N],
                    in_=adjm[:, jt * N : (jt + 1) * N],
                )
            sb_proj = consts.tile([128, NT * WC], BF16)
            sb_src = consts.tile([128, NT * H], F32)
            sb_dstB = consts.tile([128, H * N], F32)
            out_sb = consts.tile([128, N], F32)

            # --- precompute: proj'' (with bias + ones col) and src scores ---
            for jt in range(NT):
                pp = prep.tile([128, 512], F32, tag="pre")
                nc.tensor.matmul(
                    pp[:, : WC + H],
                    lhsT=sb_xT[:, jt * 128 : (jt + 1) * 128],
                    rhs=sb_wcat[:, :],
                    start=True,
                    stop=True,
                )
                nc.vector.tensor_copy(
                    out=sb_proj[:, jt * WC : (jt + 1) * WC], in_=pp[:, :WC]
                )
                nc.vector.tensor_copy(
                    out=sb_src[:, jt * H : (jt + 1) * H], in_=pp[:, WC : WC + H]
                )

            # --- dst scores broadcast across partitions: lhsT columns are the
            # same a_dst vector repeated 128x, so out[m, n] = dst[n] for all m.
            for h in range(H):
                for half in range(2):
                    pb = prep.tile([128, 512], F32, tag="pre")
                    nc.tensor.matmul(
                        pb[:, :],
                        lhsT=sb_adstb[:, h * 128 : (h + 1) * 128],
                        rhs=sb_xT[:, half * 512 : (half + 1) * 512],
                        start=True,
                        stop=True,
                    )
                    nc.vector.tensor_copy(
                        out=sb_dstB[:, h * N + half * 512 : h * N + (half + 1) * 512],
                        in_=pb[:, :],
                    )

            # --- main loop: per (head, j-tile) build Et [128 j, 1024 i] ---
            accs = [
                accp.tile([128, 2 * WC], F32, tag=f"acc{i}", name=f"acc{i}")
                for i in range(4)
            ]
            for h in range(H):
                atts = []
                for jt in range(NT):
                    sc = sb_src[:, jt * H + h : jt * H + h + 1]
                    pc = sb_pcol[:, jt * H + h : jt * H + h + 1]
                    tm = tmp.tile([128, N], F32, tag="tm")
                    # tm = dst_i (bcast rows) + src_j (per-partition) + mask
                    nc.vector.scalar_tensor_tensor(
                        out=tm,
                        in0=sb_dstB[:, h * N : (h + 1) * N],
                        scalar=sc,
                        in1=sb_adjm[:, jt * N : (jt + 1) * N],
                        op0=Add,
                        op1=Add,
                    )
                    e1 = epool.tile([128, N], BF16, tag="e1")
                    nc.scalar.activation(out=e1, in_=tm, func=Exp, bias=pc, scale=1.0)
                    e2 = epool.tile([128, N], BF16, tag="e2")
                    nc.scalar.activation(out=e2, in_=tm, func=Exp, bias=pc, scale=0.2)
                    att = attp.tile([128, N], BF16, tag="att", name=f"att_h{h}_j{jt}")
                    nc.vector.tensor_max(out=att, in0=e1, in1=e2)
                    atts.append(att)
                # it-major, jt reversed: the group leader reads the
                # latest-finishing att tile, so its single DVE wait covers
                # every att and the remaining matmuls carry no sync waits
                # (walrus LW structs have very few wait slots).
                for it in range(NT):
                    acc = accs[it // 2]
                    cb = (it % 2) * WC + h * OC
                    for jt in reversed(range(NT)):
                        nc.tensor.matmul(
                            acc[:, cb : cb + OC],
                            lhsT=atts[jt][:, it * 128 : (it + 1) * 128],
                            rhs=sb_proj[:, jt * WC + h * OC : jt * WC + (h + 1) * OC],
                            start=(jt == NT - 1),
                            stop=(jt == 0),
                        )

            # --- finalize: divide by the ones-column sum, store ---
            for it in range(NT):
                for h in range(H):
                    acc = accs[it // 2]
                    cb = (it % 2) * WC + h * OC
                    d = tmp.tile([128, 1], F32, tag="d")
                    nc.vector.reciprocal(out=d, in_=acc[:, cb + O : cb + O + 1])
                    nc.vector.tensor_scalar_mul(
                        out=out_sb[:, it * 128 + h * O : it * 128 + (h + 1) * O],
                        in0=acc[:, cb : cb + O],
                        scalar1=d,
                    )
            for it in range(NT):
                nc.sync.dma_start(
                    out=out[it * 128 : (it + 1) * 128, :],
                    in_=out_sb[:, it * 128 : (it + 1) * 128],
                )
    nc.finalize()
    return nc


def _get_nc():
    if "nc" not in _NC_CACHE:
        _NC_CACHE["nc"] = _build_nc()
    return _NC_CACHE["nc"]


def _prep_inputs(x, adj, source_prior, beta, weight, attn_src, attn_dst, bias):
    x = np.asarray(x, np.float32)
    adj = np.asarray(adj)
    source_prior = np.asarray(source_prior, np.float32)
    beta = np.asarray(beta, np.float32)
    weight = np.asarray(weight, np.float32)
    attn_src = np.asarray(attn_src, np.float32)
    attn_dst = np.asarray(attn_dst, np.float32)
    bias = np.asarray(bias, np.float32)

    # additive mask, transposed (source j on rows), tiled to [128, NT*N]
    madd = np.where(adj.T != 0, np.float32(0.0), np.float32(NEG_BIG))
    adjm = np.ascontiguousarray(
        madd.reshape(NT, 128, N).transpose(1, 0, 2).reshape(128, NT * N)
    )

    WC = H * OC
    wcat = np.zeros((IDIM + 1, WC + H), np.float32)
    adstb = np.zeros((IDIM + 1, H * 128), np.float32)
    for h in range(H):
        wcat[:IDIM, h * OC : h * OC + O] = weight[h]
        wcat[IDIM, h * OC : h * OC + O] = bias[h]
        wcat[IDIM, h * OC + O] = 1.0  # ones column -> softmax denominator
        wcat[:IDIM, WC + h] = weight[h] @ attn_src[h]
        wcat[IDIM, WC + h] = bias[h] @ attn_src[h]
        a_dst = np.concatenate([weight[h] @ attn_dst[h], bias[h] @ attn_dst[h][:, None]])
        adstb[:, h * 128 : (h + 1) * 128] = a_dst[:, None]

    gain = np.logaddexp(0.0, beta).astype(np.float32)  # softplus

    in_maps = []
    for b in range(B):
        xT = np.ones((IDIM + 1, N), np.float32)
        xT[:IDIM] = x[b].T
        cst = np.ascontiguousarray(np.concatenate([xT, wcat, adstb], axis=1))
        p = gain[None, :] * source_prior[b][:, None]  # [N, H]
        pcol = np.ascontiguousarray(
            p.reshape(NT, 128, H).transpose(1, 0, 2).reshape(128, NT * H)
        )
        in_maps.append({"cst": cst, "adjm": adjm, "pcol": pcol})
    return in_maps


def _run(inputs, trace=False):
    in_maps = _prep_inputs(**inputs)
    nc = _get_nc()
    res = run_bass_kernel_spmd(nc, in_maps, list(range(B)), trace=trace)
    out = np.stack([res.results[b]["out"] for b in range(B)]).astype(np.float32)
    return out, res


def kernel(**inputs):
    out, _ = _run(inputs, trace=False)
    return out
